# revision 16
# baseline (speedup 1.0000x reference)
"""Trainium2 Bass kernel for a 3x3 conv (N=32, C_in=128, H=W=56, C_out=256,
stride 1, pad 1), data-parallel over batch across 8 NeuronCores.

Strategy: shift-and-accumulate conv-as-matmul. The padded input image lives
in SBUF as [C_in=128 partitions, 58, 58]. For each of the 9 kernel offsets
(kh, kw) the matmul reads a shifted window of the image as a strided access
pattern [128, 8 rows, 56 cols] (N=448 moving elements) and accumulates
lhsT.T @ rhs into PSUM, where lhsT = weights[(kh,kw)] laid out [C_in, C_out].
C_out=256 is split into two chunks of 128 (PSUM partition limit). Inputs are
fed to the PE as float32r, which streams at 1 column/cycle for N>=256
(4x faster than plain fp32 matmul).
"""

import numpy as np

import concourse.bacc as bacc
import concourse.mybir as mybir
from concourse.tile import TileContext
from concourse.bass_utils import run_bass_kernel_spmd

N, C_IN, H, W = 32, 128, 56, 56
C_OUT, KS = 256, 3
N_CORES = 8
N_PER = N // N_CORES          # images per core
HP, WP = H + 2, W + 2         # zero-padded image dims
ROWS = 8                      # output rows per matmul tile
NT = ROWS * W                 # moving free dim per matmul (448 <= 512)
N_HB = H // ROWS              # row blocks per image
N_CH = C_OUT // 128           # C_out chunks
BAND_EDGES = [0, 10, 26, 42, HP]  # input DMA row bands (first band small
                                  # so the first matmul group starts early)

F32 = mybir.dt.float32
F32R = mybir.dt.float32r

_NC_CACHE = None


def _build():
    nc = bacc.Bacc(None, target_bir_lowering=False)
    x = nc.dram_tensor("x", [N_PER, C_IN, HP, WP], F32R, kind="ExternalInput")
    w = nc.dram_tensor("w", [N_CH, C_IN, KS * KS, 128], F32R, kind="ExternalInput")
    out = nc.dram_tensor("out", [N_PER, N_CH, 128, H, W], F32, kind="ExternalOutput")

    band_edges = BAND_EDGES

    with TileContext(nc) as tc:
        with (
            tc.tile_pool(name="wpool", bufs=1) as wpool,
            tc.tile_pool(name="imgs", bufs=1) as imgs,
            tc.tile_pool(name="outs", bufs=6) as outs,
            tc.tile_pool(name="psum", bufs=8, space="PSUM") as psum_pool,
        ):
            # scratch operands for PE pre-warm matmuls (content irrelevant)
            warm_sb = wpool.tile([C_IN, 128], F32, name="warm", tag="warm")
            nc.gpsimd.memset(warm_sb[:, :], 0)

            w_sbs = []
            for ch in range(N_CH):
                w_sb = wpool.tile(
                    [C_IN, KS * KS * 128], F32R, name=f"w{ch}", tag=f"w{ch}"
                )
                w_sbs.append(w_sb)
            x_sbs = []
            for n in range(N_PER):
                x_sb = imgs.tile([C_IN, HP, WP], F32R, name=f"img{n}", tag=f"img{n}")
                x_sbs.append(x_sb)

            # DMA issue order: first-needed first. ch0 weights, then image 0
            # band 0, then ch1 weights, then the remaining bands (image-major
            # band order so image 0 completes first).
            w0_flat = w.ap()[0].rearrange("c k o -> c (k o)")
            nc.sync.dma_start(w_sbs[0][:, : 3 * 128], w0_flat[:, : 3 * 128])
            r0, r1 = band_edges[0], band_edges[1]
            nc.sync.dma_start(x_sbs[0][:, r0:r1, :], x.ap()[0][:, r0:r1, :])
            nc.sync.dma_start(w_sbs[0][:, 3 * 128 :], w0_flat[:, 3 * 128 :])
            nc.sync.dma_start(
                w_sbs[1][:, :], w.ap()[1].rearrange("c k o -> c (k o)")
            )
            for n in range(N_PER):
                for b in range(len(band_edges) - 1):
                    if n == 0 and b == 0:
                        continue
                    r0, r1 = band_edges[b], band_edges[b + 1]
                    nc.sync.dma_start(
                        x_sbs[n][:, r0:r1, :], x.ap()[n][:, r0:r1, :]
                    )

            # PE pre-warm: dense dummy matmuls while the input DMAs land, so
            # the HAM clock-gate reaches K=8/8 before the real matmuls start.
            # ~40 x 128-col matmuls ~= 6us of PE activity from t~=6us.
            warm_ps = psum_pool.tile([128, 128], F32, name="warm_ps", tag="ps0")
            for _ in range(16):
                nc.tensor.matmul(
                    warm_ps[:, :], warm_sb[:, :], warm_sb[:, :],
                    start=True, stop=True,
                )

            for n in range(N_PER):
                for ch in range(N_CH):
                    for hb in range(N_HB):
                        h0 = hb * ROWS
                        ps = psum_pool.tile([128, NT], F32, name="ps", tag="ps0")
                        for i in range(KS * KS):
                            kh, kw = divmod(i, KS)
                            lhsT = w_sbs[ch][:, i * 128 : (i + 1) * 128]
                            rhs = x_sbs[n][
                                :, h0 + kh : h0 + kh + ROWS, kw : kw + W
                            ]
                            nc.tensor.matmul(
                                ps[:, :], lhsT, rhs,
                                start=(i == 0), stop=(i == KS * KS - 1),
                            )
                        o_sb = outs.tile([128, NT], F32, name="osb", tag="osb")
                        nc.vector.tensor_copy(o_sb[:, :], ps[:, :])
                        dst = out.ap()[n][ch][:, h0 : h0 + ROWS, :]
                        nc.sync.dma_start(dst, o_sb[:, :])
    nc.compile()
    return nc


def _get_nc():
    global _NC_CACHE
    if _NC_CACHE is None:
        _NC_CACHE = _build()
    return _NC_CACHE


def _prep_inputs(x: np.ndarray, kernels: np.ndarray):
    x = np.ascontiguousarray(x, dtype=np.float32)
    kernels = np.ascontiguousarray(kernels, dtype=np.float32)
    x_pad = np.zeros((N, C_IN, HP, WP), dtype=np.float32)
    x_pad[:, :, 1 : 1 + H, 1 : 1 + W] = x
    # (C_out, C_in, kh, kw) -> (ch, C_in, kh*kw, 128)
    w_t = np.ascontiguousarray(
        kernels.reshape(N_CH, 128, C_IN, KS * KS).transpose(0, 2, 3, 1)
    )
    in_maps = []
    for c in range(N_CORES):
        in_maps.append({
            "x": x_pad[c * N_PER : (c + 1) * N_PER],
            "w": w_t,
        })
    return in_maps


def _run(x: np.ndarray, kernels: np.ndarray, **kwargs):
    nc = _get_nc()
    in_maps = _prep_inputs(x, kernels)
    res = run_bass_kernel_spmd(nc, in_maps, core_ids=list(range(N_CORES)), **kwargs)
    out = np.concatenate(
        [r["out"].reshape(N_PER, C_OUT, H, W) for r in res.results], axis=0
    )
    return out, res


def kernel(x: np.ndarray, kernels: np.ndarray) -> np.ndarray:
    out, _ = _run(x, kernels)
    return out


# revision 17
# speedup vs baseline: 1.0048x; 1.0048x over previous
"""Trainium2 Bass kernel for a 3x3 conv (N=32, C_in=128, H=W=56, C_out=256,
stride 1, pad 1), data-parallel over batch across 8 NeuronCores.

Strategy: shift-and-accumulate conv-as-matmul. The padded input image lives
in SBUF as [C_in=128 partitions, 58, 58]. For each of the 9 kernel offsets
(kh, kw) the matmul reads a shifted window of the image as a strided access
pattern [128, 8 rows, 56 cols] (N=448 moving elements) and accumulates
lhsT.T @ rhs into PSUM, where lhsT = weights[(kh,kw)] laid out [C_in, C_out].
C_out=256 is split into two chunks of 128 (PSUM partition limit). Inputs are
fed to the PE as float32r, which streams at 1 column/cycle for N>=256
(4x faster than plain fp32 matmul).
"""

import numpy as np

import concourse.bacc as bacc
import concourse.mybir as mybir
from concourse.tile import TileContext
from concourse.bass_utils import run_bass_kernel_spmd

N, C_IN, H, W = 32, 128, 56, 56
C_OUT, KS = 256, 3
N_CORES = 8
N_PER = N // N_CORES          # images per core
HP, WP = H + 2, W + 2         # zero-padded image dims
ROWS = 8                      # output rows per matmul tile
NT = ROWS * W                 # moving free dim per matmul (448 <= 512)
N_HB = H // ROWS              # row blocks per image
N_CH = C_OUT // 128           # C_out chunks
BAND_EDGES = [0, 10, 26, 42, HP]  # input DMA row bands (first band small
                                  # so the first matmul group starts early)

F32 = mybir.dt.float32
F32R = mybir.dt.float32r

_NC_CACHE = None


def _build():
    nc = bacc.Bacc(None, target_bir_lowering=False)
    x = nc.dram_tensor("x", [N_PER, C_IN, HP, WP], F32R, kind="ExternalInput")
    w = nc.dram_tensor("w", [N_CH, C_IN, KS * KS, 128], F32R, kind="ExternalInput")
    out = nc.dram_tensor("out", [N_PER, N_CH, 128, H, W], F32, kind="ExternalOutput")

    band_edges = BAND_EDGES

    with TileContext(nc) as tc:
        with (
            tc.tile_pool(name="wpool", bufs=1) as wpool,
            tc.tile_pool(name="imgs", bufs=1) as imgs,
            tc.tile_pool(name="outs", bufs=6) as outs,
            tc.tile_pool(name="psum", bufs=8, space="PSUM") as psum_pool,
        ):
            # scratch operands for PE pre-warm matmuls (content irrelevant)
            warm_sb = wpool.tile([C_IN, 128], F32, name="warm", tag="warm")
            nc.gpsimd.memset(warm_sb[:, :], 0)

            w_sbs = []
            for ch in range(N_CH):
                w_sb = wpool.tile(
                    [C_IN, KS * KS * 128], F32R, name=f"w{ch}", tag=f"w{ch}"
                )
                w_sbs.append(w_sb)
            x_sbs = []
            for n in range(N_PER):
                x_sb = imgs.tile([C_IN, HP, WP], F32R, name=f"img{n}", tag=f"img{n}")
                x_sbs.append(x_sb)

            # DMA issue order: first-needed first. ch0 weights, then image 0
            # band 0, then ch1 weights, then the remaining bands (image-major
            # band order so image 0 completes first).
            w0_flat = w.ap()[0].rearrange("c k o -> c (k o)")
            nc.sync.dma_start(w_sbs[0][:, : 3 * 128], w0_flat[:, : 3 * 128])
            r0, r1 = band_edges[0], band_edges[1]
            nc.sync.dma_start(x_sbs[0][:, r0:r1, :], x.ap()[0][:, r0:r1, :])
            nc.sync.dma_start(w_sbs[0][:, 3 * 128 :], w0_flat[:, 3 * 128 :])
            nc.sync.dma_start(
                w_sbs[1][:, :], w.ap()[1].rearrange("c k o -> c (k o)")
            )
            for n in range(N_PER):
                for b in range(len(band_edges) - 1):
                    if n == 0 and b == 0:
                        continue
                    r0, r1 = band_edges[b], band_edges[b + 1]
                    nc.sync.dma_start(
                        x_sbs[n][:, r0:r1, :], x.ap()[n][:, r0:r1, :]
                    )

            # PE pre-warm: dense dummy matmuls while the input DMAs land, so
            # the HAM clock-gate reaches K=8/8 before the real matmuls start
            # (16 fp32 matmuls = 32 HW matmuls ~= 4us of PE activity).
            warm_ps = psum_pool.tile([128, 128], F32, name="warm_ps", tag="ps0")
            for _ in range(16):
                nc.tensor.matmul(
                    warm_ps[:, :], warm_sb[:, :], warm_sb[:, :],
                    start=True, stop=True,
                )

            for n in range(N_PER):
                for ch in range(N_CH):
                    for hb in range(N_HB):
                        h0 = hb * ROWS
                        ps = psum_pool.tile([128, NT], F32, name="ps", tag="ps0")
                        for i in range(KS * KS):
                            kh, kw = divmod(i, KS)
                            lhsT = w_sbs[ch][:, i * 128 : (i + 1) * 128]
                            rhs = x_sbs[n][
                                :, h0 + kh : h0 + kh + ROWS, kw : kw + W
                            ]
                            nc.tensor.matmul(
                                ps[:, :], lhsT, rhs,
                                start=(i == 0), stop=(i == KS * KS - 1),
                            )
                        o_sb = outs.tile([128, NT], F32, name="osb", tag="osb")
                        nc.vector.tensor_copy(o_sb[:, :], ps[:, :])
                        dst = out.ap()[n][ch][:, h0 : h0 + ROWS, :]
                        nc.sync.dma_start(dst, o_sb[:, :])
    nc.compile()
    return nc


def _get_nc():
    global _NC_CACHE
    if _NC_CACHE is None:
        _NC_CACHE = _build()
    return _NC_CACHE


def _prep_inputs(x: np.ndarray, kernels: np.ndarray):
    x = np.ascontiguousarray(x, dtype=np.float32)
    kernels = np.ascontiguousarray(kernels, dtype=np.float32)
    x_pad = np.zeros((N, C_IN, HP, WP), dtype=np.float32)
    x_pad[:, :, 1 : 1 + H, 1 : 1 + W] = x
    # (C_out, C_in, kh, kw) -> (ch, C_in, kh*kw, 128)
    w_t = np.ascontiguousarray(
        kernels.reshape(N_CH, 128, C_IN, KS * KS).transpose(0, 2, 3, 1)
    )
    in_maps = []
    for c in range(N_CORES):
        in_maps.append({
            "x": x_pad[c * N_PER : (c + 1) * N_PER],
            "w": w_t,
        })
    return in_maps


def _run(x: np.ndarray, kernels: np.ndarray, **kwargs):
    nc = _get_nc()
    in_maps = _prep_inputs(x, kernels)
    res = run_bass_kernel_spmd(nc, in_maps, core_ids=list(range(N_CORES)), **kwargs)
    out = np.concatenate(
        [r["out"].reshape(N_PER, C_OUT, H, W) for r in res.results], axis=0
    )
    return out, res


def kernel(x: np.ndarray, kernels: np.ndarray) -> np.ndarray:
    out, _ = _run(x, kernels)
    return out


# revision 20
# speedup vs baseline: 1.0268x; 1.0219x over previous
"""Trainium2 Bass kernel for a 3x3 conv (N=32, C_in=128, H=W=56, C_out=256,
stride 1, pad 1), data-parallel over batch across 8 NeuronCores.

Strategy: shift-and-accumulate conv-as-matmul. The padded input image lives
in SBUF as [C_in=128 partitions, 58, 58]. For each of the 9 kernel offsets
(kh, kw) the matmul reads a shifted window of the image as a strided access
pattern [128, 8 rows, 56 cols] (N=448 moving elements) and accumulates
lhsT.T @ rhs into PSUM, where lhsT = weights[(kh,kw)] laid out [C_in, C_out].
C_out=256 is split into two chunks of 128 (PSUM partition limit). Inputs are
fed to the PE as float32r, which streams at 1 column/cycle for N>=256
(4x faster than plain fp32 matmul).
"""

import numpy as np

import concourse.bacc as bacc
import concourse.mybir as mybir
from concourse.tile import TileContext
from concourse.bass_utils import run_bass_kernel_spmd

N, C_IN, H, W = 32, 128, 56, 56
C_OUT, KS = 256, 3
N_CORES = 8
N_PER = N // N_CORES          # images per core
HP, WP = H + 2, W + 2         # zero-padded image dims
ROWS = 8                      # output rows per matmul tile
NT = ROWS * W                 # moving free dim per matmul (448 <= 512)
N_HB = H // ROWS              # row blocks per image
N_CH = C_OUT // 128           # C_out chunks
BAND_EDGES = [0, 10, 26, 42, HP]  # input DMA row bands (first band small
                                  # so the first matmul group starts early)

F32 = mybir.dt.float32
F32R = mybir.dt.float32r

_NC_CACHE = None


def _build():
    nc = bacc.Bacc(None, target_bir_lowering=False)
    x = nc.dram_tensor("x", [N_PER, C_IN, HP, WP], F32R, kind="ExternalInput")
    w = nc.dram_tensor("w", [N_CH, C_IN, KS * KS, 128], F32R, kind="ExternalInput")
    out = nc.dram_tensor("out", [N_PER, N_CH, 128, H, W], F32, kind="ExternalOutput")

    band_edges = BAND_EDGES

    with TileContext(nc) as tc:
        with (
            tc.tile_pool(name="wpool", bufs=1) as wpool,
            tc.tile_pool(name="imgs", bufs=1) as imgs,
            tc.tile_pool(name="outs", bufs=6) as outs,
            tc.tile_pool(name="psum", bufs=8, space="PSUM") as psum_pool,
        ):
            # scratch operands for PE pre-warm matmuls (content irrelevant)
            warm_sb = wpool.tile([C_IN, 128], F32, name="warm", tag="warm")
            nc.gpsimd.memset(warm_sb[:, :], 0)

            w_sbs = []
            for ch in range(N_CH):
                w_sb = wpool.tile(
                    [C_IN, KS * KS * 128], F32R, name=f"w{ch}", tag=f"w{ch}"
                )
                w_sbs.append(w_sb)
            x_sbs = []
            for n in range(N_PER):
                x_sb = imgs.tile([C_IN, HP, WP], F32R, name=f"img{n}", tag=f"img{n}")
                x_sbs.append(x_sb)

            # DMA issue order: first-needed first, split across the two HWDGE
            # engines so descriptor generation runs in parallel — weights on
            # Sync while image 0's first band issues on Scalar. Output stores
            # also go on Scalar (it is otherwise idle), keeping Sync free for
            # the remaining input bands.
            w0_flat = w.ap()[0].rearrange("c k o -> c (k o)")
            nc.sync.dma_start(w_sbs[0][:, : 3 * 128], w0_flat[:, : 3 * 128])
            r0, r1 = band_edges[0], band_edges[1]
            nc.scalar.dma_start(x_sbs[0][:, r0:r1, :], x.ap()[0][:, r0:r1, :])
            nc.sync.dma_start(w_sbs[0][:, 3 * 128 :], w0_flat[:, 3 * 128 :])
            nc.sync.dma_start(
                w_sbs[1][:, :], w.ap()[1].rearrange("c k o -> c (k o)")
            )
            for n in range(N_PER):
                for b in range(len(band_edges) - 1):
                    if n == 0 and b == 0:
                        continue
                    r0, r1 = band_edges[b], band_edges[b + 1]
                    nc.sync.dma_start(
                        x_sbs[n][:, r0:r1, :], x.ap()[n][:, r0:r1, :]
                    )

            # PE pre-warm: dense dummy matmuls while the input DMAs land, so
            # the HAM clock-gate reaches K=8/8 before the real matmuls start
            # (16 fp32 matmuls = 32 HW matmuls ~= 4us of PE activity).
            warm_ps = psum_pool.tile([128, 128], F32, name="warm_ps", tag="ps0")
            for _ in range(12):
                nc.tensor.matmul(
                    warm_ps[:, :], warm_sb[:, :], warm_sb[:, :],
                    start=True, stop=True,
                )

            for n in range(N_PER):
                for ch in range(N_CH):
                    for hb in range(N_HB):
                        h0 = hb * ROWS
                        ps = psum_pool.tile([128, NT], F32, name="ps", tag="ps0")
                        for i in range(KS * KS):
                            kh, kw = divmod(i, KS)
                            lhsT = w_sbs[ch][:, i * 128 : (i + 1) * 128]
                            rhs = x_sbs[n][
                                :, h0 + kh : h0 + kh + ROWS, kw : kw + W
                            ]
                            nc.tensor.matmul(
                                ps[:, :], lhsT, rhs,
                                start=(i == 0), stop=(i == KS * KS - 1),
                            )
                        o_sb = outs.tile([128, NT], F32, name="osb", tag="osb")
                        nc.vector.tensor_copy(o_sb[:, :], ps[:, :])
                        dst = out.ap()[n][ch][:, h0 : h0 + ROWS, :]
                        nc.scalar.dma_start(dst, o_sb[:, :])
    nc.compile()
    return nc


def _get_nc():
    global _NC_CACHE
    if _NC_CACHE is None:
        _NC_CACHE = _build()
    return _NC_CACHE


def _prep_inputs(x: np.ndarray, kernels: np.ndarray):
    x = np.ascontiguousarray(x, dtype=np.float32)
    kernels = np.ascontiguousarray(kernels, dtype=np.float32)
    x_pad = np.zeros((N, C_IN, HP, WP), dtype=np.float32)
    x_pad[:, :, 1 : 1 + H, 1 : 1 + W] = x
    # (C_out, C_in, kh, kw) -> (ch, C_in, kh*kw, 128)
    w_t = np.ascontiguousarray(
        kernels.reshape(N_CH, 128, C_IN, KS * KS).transpose(0, 2, 3, 1)
    )
    in_maps = []
    for c in range(N_CORES):
        in_maps.append({
            "x": x_pad[c * N_PER : (c + 1) * N_PER],
            "w": w_t,
        })
    return in_maps


def _run(x: np.ndarray, kernels: np.ndarray, **kwargs):
    nc = _get_nc()
    in_maps = _prep_inputs(x, kernels)
    res = run_bass_kernel_spmd(nc, in_maps, core_ids=list(range(N_CORES)), **kwargs)
    out = np.concatenate(
        [r["out"].reshape(N_PER, C_OUT, H, W) for r in res.results], axis=0
    )
    return out, res


def kernel(x: np.ndarray, kernels: np.ndarray) -> np.ndarray:
    out, _ = _run(x, kernels)
    return out
